# revision 10
# baseline (speedup 1.0000x reference)
"""DistMult edge scoring on 8 Trainium2 NeuronCores.

score[r, e] = sum_d x[src[r,e], d] * x[dst[r,e], d] * rel[r, d]

Strategy (edge-sharded, data-parallel):
  - Shard the 500k edges across 8 cores (62500 each); replicate the node
    table and rel embeddings in each core's DRAM, converted to fp16 on
    the host (rel-err ~5e-4 << 2e-2).
  - Gathers use the batched GPSIMD dma_gather spread round-robin over all
    4 SWDGE queues with a 9-deep tile pool so gathers stay in flight.
  - ALIGN512: descriptors fetch 512 B (an aligned PAIR of table rows)
    instead of 256 B. HW-measured on this part, 512 B descriptors move
    ~2.5-3x more bytes/s than 256 B ones (256 B pays a small-transfer
    penalty), so fetching 2 rows and using 1 is a large net win. The
    edge's own row sits at pair-offset (row % 2); buckets are split by
    the parity of each side so every chunk reads a uniform 128-elem
    slice of its 256-elem slots (strided DVE APs, no data-dependent
    addressing). Aligned pair-tokens also double int16 index reach to
    65536 rows, so each side needs only 2 super-ranges; bucket key =
    (src_super, src_parity, dst_super, dst_parity) — 16 buckets as
    before.
  - Buckets are padded to the max size across cores (rounded to 128) so a
    single SPMD program serves all 8 cores; pad slots gather token 0 of
    the range and their scores are discarded on the host.
  - The src side gathers from 3 host-premultiplied rel-scaled tables
    (tabr[r] = table * rel[r]), so per chunk of ~2048 edges the DVE does
    just v = xs_rel * xd and a grouped reduce_sum -> scores.
  - Edges are dst-sorted within each bucket (sorted ascending 512 B
    descriptors merge into larger bursts on the xd stream); chunk
    element t lands at (partition t%128, column t//128); the host
    inverts the whole permutation when assembling the output.
"""

import numpy as np

N_CORES = 8
N_NODES = 100000
DIM = 128
N_REL = 3
N_EDGES = 500000
E_CORE = N_EDGES // N_CORES          # 62500
RANGE = 32768                        # int16-addressable table range
N_RANGES = (N_NODES + RANGE - 1) // RANGE   # 4
N_BUCKETS = N_RANGES * N_RANGES      # 16
CHUNK_COLS = 16                      # max columns (128 edges each) per gather
GATHER_BUFS = 9                      # buffering depth for gather tiles
N_QUEUES = 4                         # SWDGE queues to spread gathers over
SINGLE_PACKET = False                # True hangs the device — keep False
GATHER_DTYPE = "float16"             # float32 | float16 (table+gather payload)
REL_FOLDED = True                    # src side gathers from rel-scaled tables
QUEUE_MODE = "rr"                    # rr: global round-robin; side: xs=q0/1, xd=q2/3
GROUP_CHUNKS = 0                     # chunks per stream-run group; 0 = GATHER_BUFS-1
MERGE_SRC = 0                        # merge xs gathers of N buckets sharing s_tab (0/2)

ALIGN512 = True  # 512 B gather descriptors (2 rows/desc, parity-split buckets)
SUPER = 65536    # rows reachable by int16 pair-tokens (32768 x 512 B)
N_SUPER = 2      # ceil(N_NODES / SUPER)

_CACHE = {}
LAST_RESULTS = None  # bass_utils.BassKernelResults from the most recent run


def _build_nc(caps, cols_max, body_reps=1):
    """caps[r][k] = padded bucket capacity (multiple of 128) for relation r,
    bucket k = src_range*N_RANGES + dst_range. cols_max = max_r total cols."""
    from contextlib import ExitStack

    import concourse.bacc as bacc
    import concourse.mybir as mybir
    import concourse.tile as tile

    tot_s = sum(sum(c) for c in caps) // 16  # int16 image cols per side
    dt_g = getattr(mybir.dt, GATHER_DTYPE)
    assert not MERGE_SRC or REL_FOLDED  # merge path has no relb multiply

    nc = bacc.Bacc("TRN2", target_bir_lowering=False, debug=False,
                   num_devices=N_CORES, num_swdge_queues=N_QUEUES)
    table = nc.dram_tensor("table", (N_NODES, DIM), dt_g,
                           kind="ExternalInput")
    if REL_FOLDED:
        tabr = nc.dram_tensor("tabr", (N_REL, N_NODES, DIM), dt_g,
                              kind="ExternalInput")
    else:
        relb = nc.dram_tensor("relb", (N_REL, 128, DIM), dt_g,
                              kind="ExternalInput")
    sidx = nc.dram_tensor("sidx", (128, tot_s), mybir.dt.int16,
                          kind="ExternalInput")
    didx = nc.dram_tensor("didx", (128, tot_s), mybir.dt.int16,
                          kind="ExternalInput")
    out = nc.dram_tensor("out", (N_REL, 128, cols_max), mybir.dt.float32,
                         kind="ExternalOutput")
    qn = [0]  # round-robin SWDGE queue assignment state

    with tile.TileContext(nc) as tc, ExitStack() as ctx:
        const_pool = ctx.enter_context(tc.tile_pool(name="const", bufs=2))
        ipool = ctx.enter_context(tc.tile_pool(name="idx", bufs=1))
        if MERGE_SRC:
            xpool = ctx.enter_context(tc.tile_pool(name="xsm", bufs=5))
            dpool = ctx.enter_context(tc.tile_pool(name="xdm", bufs=7))
        else:
            gpool = ctx.enter_context(tc.tile_pool(name="gather",
                                                   bufs=GATHER_BUFS))
        spool = ctx.enter_context(tc.tile_pool(name="scores", bufs=2))

        def next_q(side=None):
            if QUEUE_MODE == "side" and N_QUEUES == 4 and side is not None:
                base = 0 if side == "s" else 2
                q = base + qn[0] % 2
                qn[0] += 1
                return q
            q = qn[0]
            qn[0] = (q + 1) % N_QUEUES
            return q

        # whole int16 index images stay SBUF-resident (~3 MB per side)
        sidx_sb = ipool.tile([128, tot_s], mybir.dt.int16, tag="sidx")
        didx_sb = ipool.tile([128, tot_s], mybir.dt.int16, tag="didx")
        nc.sync.dma_start(out=sidx_sb[:], in_=sidx[:, :])
        nc.sync.dma_start(out=didx_sb[:], in_=didx[:, :])

        for _rep in range(body_reps):
            img_off = 0  # running column offset into the int16 index images
            for r in range(N_REL):
                if not REL_FOLDED:
                    relb_sb = const_pool.tile([128, DIM], dt_g, tag="relb")
                    nc.sync.dma_start(out=relb_sb[:], in_=relb[r, :, :])
                cols_r = sum(caps[r]) // 128
                scores_sb = spool.tile([128, cols_max], mybir.dt.float32,
                                       tag="scores")
                # flatten the chunk list across buckets so the stream-run
                # grouping below actually gets GATHER_BUFS-1 chunks per
                # group (a single bucket is only ~31 cols = 1 chunk)
                col_off = 0
                chunks = []
                for k in range(N_BUCKETS):
                    cap = caps[r][k]
                    if cap == 0:
                        continue
                    if ALIGN512:
                        ks, sp = k >> 3, (k >> 2) & 1
                        kd, dp = (k >> 1) & 1, k & 1
                        ssi, dsi = ks * SUPER, kd * SUPER
                        s_tab = tabr[r, ssi : min(ssi + SUPER, N_NODES), :] \
                            .rearrange("(t e) d -> t (e d)", e=2)
                        d_tab = table[dsi : min(dsi + SUPER, N_NODES), :] \
                            .rearrange("(t e) d -> t (e d)", e=2)
                    else:
                        sp = dp = 0
                        si = (k // N_RANGES) * RANGE
                        di = (k % N_RANGES) * RANGE
                        if REL_FOLDED:
                            s_tab = tabr[r, si : min(si + RANGE, N_NODES), :]
                        else:
                            s_tab = table[si : min(si + RANGE, N_NODES), :]
                        d_tab = table[di : min(di + RANGE, N_NODES), :]
                    cols_b = cap // 128
                    for c0 in range(0, cols_b, CHUNK_COLS):
                        cc = min(CHUNK_COLS, cols_b - c0)
                        chunks.append((cc, img_off, s_tab, d_tab, col_off,
                                       sp, dp))
                        img_off += cc * 8
                        col_off += cc
                assert col_off == cols_r
                # issue gathers grouped by stream (src runs, then dst runs):
                # back-to-back same-stream gathers beat alternating src/dst.
                # Group size capped below GATHER_BUFS so the xs tiles of a
                # group can all be live at once.
                if MERGE_SRC:
                    # merge consecutive chunks sharing s_tab into one xs
                    # gather (<= 2*CHUNK_COLS cols = 8192 descs, HW-safe)
                    merged = []
                    for ch in chunks:
                        cc, io, s_tab, d_tab, co = ch
                        if (merged and merged[-1][2] is s_tab
                                and merged[-1][0] + cc <= 2 * CHUNK_COLS):
                            m = merged[-1]
                            merged[-1] = (m[0] + cc, m[1], s_tab, m[3] + [ch])
                        else:
                            merged.append((cc, io, s_tab, [ch]))
                    for g0 in range(0, len(merged), 4):
                        mg = merged[g0 : g0 + 4]
                        xs_tiles = []
                        for cct, io0, s_tab, subs in mg:
                            xs_t = xpool.tile([128, 2 * CHUNK_COLS * DIM],
                                              dt_g, tag="xsm")
                            nc.gpsimd.dma_gather(
                                xs_t[:, : cct * DIM].rearrange(
                                    "p (c d) -> p c d", d=DIM),
                                s_tab, sidx_sb[:, io0 : io0 + cct * 8],
                                cct * 128, cct * 128, DIM,
                                single_packet=SINGLE_PACKET,
                                queue_num=next_q("s"))
                            xs_tiles.append(xs_t)
                        for j, (cct, io0, s_tab, subs) in enumerate(mg):
                            xs_t = xs_tiles[j]
                            for cc, io, _st, d_tab, co in subs:
                                loc = (io - io0) // 8
                                xd_t = dpool.tile([128, CHUNK_COLS * DIM],
                                                  dt_g, tag="xd")
                                nc.gpsimd.dma_gather(
                                    xd_t[:, : cc * DIM].rearrange(
                                        "p (c d) -> p c d", d=DIM),
                                    d_tab, didx_sb[:, io : io + cc * 8],
                                    cc * 128, cc * 128, DIM,
                                    single_packet=SINGLE_PACKET,
                                    queue_num=next_q("d"))
                                nc.vector.tensor_tensor(
                                    out=xd_t[:, : cc * DIM],
                                    in0=xs_t[:, loc * DIM : (loc + cc) * DIM],
                                    in1=xd_t[:, : cc * DIM],
                                    op=mybir.AluOpType.mult)
                                nc.vector.reduce_sum(
                                    out=scores_sb[:, co : co + cc],
                                    in_=xd_t[:, : cc * DIM].rearrange(
                                        "p (c d) -> p c d", d=DIM),
                                    axis=mybir.AxisListType.X)
                    nc.sync.dma_start(out=out[r, :, :cols_r],
                                      in_=scores_sb[:, :cols_r])
                    continue
                EW = 2 * DIM if ALIGN512 else DIM  # gather elem width
                gsz = GROUP_CHUNKS or (GATHER_BUFS - 1)
                for g0 in range(0, len(chunks), gsz):
                    grp = chunks[g0 : g0 + gsz]
                    xs_tiles = []
                    for cc, io, s_tab, d_tab, co, sp, dp in grp:
                        xs_t = gpool.tile([128, CHUNK_COLS * EW],
                                          dt_g, tag="xs")
                        nc.gpsimd.dma_gather(
                            xs_t[:, : cc * EW].rearrange(
                                "p (c d) -> p c d", d=EW),
                            s_tab, sidx_sb[:, io : io + cc * 8],
                            cc * 128, cc * 128, EW,
                            single_packet=SINGLE_PACKET,
                            queue_num=next_q("s"))
                        xs_tiles.append(xs_t)
                    for j, (cc, io, s_tab, d_tab, co, sp, dp) in enumerate(grp):
                        xd_t = gpool.tile([128, CHUNK_COLS * EW],
                                          dt_g, tag="xd")
                        nc.gpsimd.dma_gather(
                            xd_t[:, : cc * EW].rearrange(
                                "p (c d) -> p c d", d=EW),
                            d_tab, didx_sb[:, io : io + cc * 8],
                            cc * 128, cc * 128, EW,
                            single_packet=SINGLE_PACKET,
                            queue_num=next_q("d"))
                        xs_t = xs_tiles[j]
                        xs3 = xs_t[:, : cc * EW].rearrange(
                            "p (c d) -> p c d", d=EW)
                        xd3 = xd_t[:, : cc * EW].rearrange(
                            "p (c d) -> p c d", d=EW)
                        prod = xs3[:, :, 0:DIM]
                        nc.vector.tensor_tensor(
                            out=prod,
                            in0=xs3[:, :, sp * DIM : (sp + 1) * DIM],
                            in1=xd3[:, :, dp * DIM : (dp + 1) * DIM],
                            op=mybir.AluOpType.mult)
                        if not REL_FOLDED:
                            nc.vector.tensor_tensor(
                                out=prod,
                                in0=prod,
                                in1=relb_sb[:, None, :].to_broadcast(
                                    [128, cc, DIM]),
                                op=mybir.AluOpType.mult)
                        nc.vector.reduce_sum(
                            out=scores_sb[:, co : co + cc],
                            in_=prod,
                            axis=mybir.AxisListType.X)
                nc.sync.dma_start(out=out[r, :, :cols_r],
                                  in_=scores_sb[:, :cols_r])

    nc.compile()
    return nc


def _pack_idx16(vals, cap):
    """Local indices [n] (n <= cap, cap % 128 == 0) -> replicated int16
    image [128, cap // 16], zero-padded."""
    a = np.zeros(cap, dtype=np.int16)
    a[: len(vals)] = vals
    return np.tile(a.reshape(-1, 16).T, (8, 1))  # [16, cap/16] -> [128, .]


SORT_MODE = "dst"  # none | src | dst | src_block_dst


def _bucket_order(s, d, b):
    """Permutation of edges grouped by bucket id b, with optional intra-bucket
    ordering for DRAM row-buffer locality on the gather streams."""
    if SORT_MODE == "none":
        return np.argsort(b, kind="stable")
    if SORT_MODE == "src":
        return np.lexsort((s, b))
    if SORT_MODE == "dst":
        return np.lexsort((d, b))
    if SORT_MODE == "dst_src":
        return np.lexsort((s, d, b))
    if SORT_MODE == "src_block_dst":
        BS = 4096  # src block rows
        return np.lexsort((d, s // BS, b))
    raise ValueError(SORT_MODE)


def prepare(node_embeds, rel_emb, src_idx, dst_idx, body_reps=1):
    """Bucket/pack host-side; returns (nc, in_maps, assemble) where
    assemble(results) -> full [N_REL, N_EDGES] scores."""
    host_dt = np.float16 if GATHER_DTYPE == "float16" else np.float32
    node32 = np.asarray(node_embeds, dtype=np.float32)
    rel32 = np.asarray(rel_emb, dtype=np.float32)
    node_embeds = np.ascontiguousarray(node32.astype(host_dt))
    rel_emb = rel32.astype(host_dt)
    src_idx = np.asarray(src_idx).astype(np.int64)
    dst_idx = np.asarray(dst_idx).astype(np.int64)

    if REL_FOLDED:
        tabr = np.ascontiguousarray(
            (node32[None, :, :] * rel32[:, None, :]).astype(host_dt))
    else:
        relb = np.ascontiguousarray(
            np.broadcast_to(rel_emb[:, None, :], (N_REL, 128, DIM)))

    # ---- host-side bucketing ----
    # orders[c][r]: edge permutation (bucket-major); counts[c][r][k]
    orders = [[None] * N_REL for _ in range(N_CORES)]
    counts = np.zeros((N_CORES, N_REL, N_BUCKETS), dtype=np.int64)
    s_loc = [[None] * N_REL for _ in range(N_CORES)]
    d_loc = [[None] * N_REL for _ in range(N_CORES)]
    for c in range(N_CORES):
        lo = c * E_CORE
        for r in range(N_REL):
            s = src_idx[r, lo : lo + E_CORE]
            d = dst_idx[r, lo : lo + E_CORE]
            if ALIGN512:
                b = ((s // SUPER) * 8 + (s % 2) * 4
                     + (d // SUPER) * 2 + (d % 2))
            else:
                b = (s // RANGE) * N_RANGES + (d // RANGE)
            order = _bucket_order(s, d, b)
            orders[c][r] = order
            counts[c, r] = np.bincount(b, minlength=N_BUCKETS)
            if ALIGN512:
                s_loc[c][r] = ((s[order] % SUPER) // 2).astype(np.int16)
                d_loc[c][r] = ((d[order] % SUPER) // 2).astype(np.int16)
            else:
                s_loc[c][r] = (s[order] % RANGE).astype(np.int16)
                d_loc[c][r] = (d[order] % RANGE).astype(np.int16)

    caps = [[int(-(-counts[:, r, k].max() // 128) * 128)
             for k in range(N_BUCKETS)] for r in range(N_REL)]
    cols_max = max(sum(caps[r]) for r in range(N_REL)) // 128

    key = (tuple(map(tuple, caps)), cols_max, body_reps, N_QUEUES,
           SINGLE_PACKET, GATHER_DTYPE, CHUNK_COLS, GATHER_BUFS, REL_FOLDED,
           QUEUE_MODE, GROUP_CHUNKS, MERGE_SRC, ALIGN512)
    if _CACHE.get("key") != key:
        _CACHE["nc"] = _build_nc(caps, cols_max, body_reps=body_reps)
        _CACHE["key"] = key
    nc = _CACHE["nc"]

    # ---- pack index images (chunked exactly like the device loop) ----
    tot_s = sum(sum(c) for c in caps) // 16
    in_maps = []
    for c in range(N_CORES):
        s_img = np.empty((128, tot_s), dtype=np.int16)
        d_img = np.empty((128, tot_s), dtype=np.int16)
        img_off = 0
        for r in range(N_REL):
            u = 0  # position within this core's bucket-sorted edge stream
            for k in range(N_BUCKETS):
                cap = caps[r][k]
                if cap == 0:
                    continue
                cnt = int(counts[c, r, k])
                sv = s_loc[c][r][u : u + cnt]
                dv = d_loc[c][r][u : u + cnt]
                u += cnt
                # pad bucket to cap, then emit in CHUNK_COLS chunks
                sp = np.zeros(cap, dtype=np.int16); sp[:cnt] = sv
                dp = np.zeros(cap, dtype=np.int16); dp[:cnt] = dv
                for c0 in range(0, cap // 128, CHUNK_COLS):
                    cc = min(CHUNK_COLS, cap // 128 - c0)
                    n = cc * 128
                    seg = slice(c0 * 128, c0 * 128 + n)
                    s_img[:, img_off : img_off + cc * 8] = _pack_idx16(sp[seg], n)
                    d_img[:, img_off : img_off + cc * 8] = _pack_idx16(dp[seg], n)
                    img_off += cc * 8
        assert img_off == tot_s
        m = {"table": node_embeds, "sidx": s_img, "didx": d_img}
        if REL_FOLDED:
            m["tabr"] = tabr
        else:
            m["relb"] = relb
        in_maps.append(m)

    def assemble(results):
        out = np.empty((N_REL, N_EDGES), dtype=np.float32)
        for c, res in enumerate(results):
            buf = res["out"]  # [N_REL, 128, cols_max]
            lo = c * E_CORE
            for r in range(N_REL):
                colmajor = buf[r].T.ravel()  # index = col*128 + partition
                u = 0
                off = 0
                for k in range(N_BUCKETS):
                    cap = caps[r][k]
                    if cap == 0:
                        continue
                    cnt = int(counts[c, r, k])
                    e_ids = orders[c][r][u : u + cnt]
                    out[r, lo + e_ids] = colmajor[off : off + cnt]
                    u += cnt
                    off += cap
        return out

    return nc, in_maps, assemble


def _spot_check(out, node_embeds, rel_emb, src_idx, dst_idx, n=256):
    """Recompute n random edges on the host (fp32) and count gross
    mismatches. Catches the rare multi-queue DMA-sem race (one nan
    warmup in ~31 runs observed): corrupted gathers read as wildly
    wrong or non-finite scores, far outside fp16 rounding (~1%)."""
    if not np.isfinite(out).all():
        return False
    rng = np.random.default_rng(0)
    x = np.asarray(node_embeds, dtype=np.float32)
    rel = np.asarray(rel_emb, dtype=np.float32)
    r = rng.integers(0, N_REL, n)
    e = rng.integers(0, N_EDGES, n)
    s = np.asarray(src_idx)[r, e].astype(np.int64)
    d = np.asarray(dst_idx)[r, e].astype(np.int64)
    exp = np.einsum("nd,nd,nd->n", x[s], x[d], rel[r])
    got = out[r, e]
    bad = np.abs(got - exp) > 0.2 * (np.abs(exp) + 1.0)
    return bad.mean() < 0.05


def kernel(node_embeds, rel_emb, src_idx, dst_idx):
    global LAST_RESULTS
    from concourse import bass_utils

    nc, in_maps, assemble = prepare(node_embeds, rel_emb, src_idx, dst_idx)
    for _attempt in range(3):
        LAST_RESULTS = bass_utils.run_bass_kernel_spmd(
            nc, in_maps, core_ids=list(range(N_CORES)))
        out = assemble(LAST_RESULTS.results)
        if _spot_check(out, node_embeds, rel_emb, src_idx, dst_idx):
            return out
    return out



# revision 11
# speedup vs baseline: 2.6282x; 2.6282x over previous
"""DistMult edge scoring on 8 Trainium2 NeuronCores.

score[r, e] = sum_d x[src[r,e], d] * x[dst[r,e], d] * rel[r, d]

Strategy (edge-sharded, data-parallel):
  - Shard the 500k edges across 8 cores (62500 each); replicate the node
    table and rel embeddings in each core's DRAM, converted to fp16 on
    the host (rel-err ~5e-4 << 2e-2).
  - Gathers use the batched GPSIMD dma_gather spread round-robin over all
    4 SWDGE queues with a 9-deep tile pool so gathers stay in flight.
  - ALIGN512: descriptors fetch 512 B (an aligned PAIR of table rows)
    instead of 256 B. HW-measured on this part, 512 B descriptors move
    ~2.5-3x more bytes/s than 256 B ones (256 B pays a small-transfer
    penalty), so fetching 2 rows and using 1 is a large net win. The
    edge's own row sits at pair-offset (row % 2); buckets are split by
    the parity of each side so every chunk reads a uniform 128-elem
    slice of its 256-elem slots (strided DVE APs, no data-dependent
    addressing). Aligned pair-tokens also double int16 index reach to
    65536 rows, so each side needs only 2 super-ranges; bucket key =
    (src_super, src_parity, dst_super, dst_parity) — 16 buckets as
    before.
  - Buckets are padded to the max size across cores (rounded to 128) so a
    single SPMD program serves all 8 cores; pad slots gather token 0 of
    the range and their scores are discarded on the host.
  - The src side gathers from 3 host-premultiplied rel-scaled tables
    (tabr[r] = table * rel[r]), so per chunk of ~2048 edges the DVE does
    just v = xs_rel * xd and a grouped reduce_sum -> scores.
  - Edges are dst-sorted within each bucket (sorted ascending 512 B
    descriptors merge into larger bursts on the xd stream); chunk
    element t lands at (partition t%128, column t//128); the host
    inverts the whole permutation when assembling the output.
"""

import numpy as np

N_CORES = 8
N_NODES = 100000
DIM = 128
N_REL = 3
N_EDGES = 500000
E_CORE = N_EDGES // N_CORES          # 62500
RANGE = 32768                        # int16-addressable table range
N_RANGES = (N_NODES + RANGE - 1) // RANGE   # 4
N_BUCKETS = N_RANGES * N_RANGES      # 16
CHUNK_COLS = 16                      # max columns (128 edges each) per gather
GATHER_BUFS = 9                      # buffering depth for gather tiles
N_QUEUES = 4                         # SWDGE queues to spread gathers over
SINGLE_PACKET = False                # True hangs the device — keep False
GATHER_DTYPE = "float16"             # float32 | float16 (table+gather payload)
REL_FOLDED = True                    # src side gathers from rel-scaled tables
QUEUE_MODE = "rr"                    # rr: global round-robin; side: xs=q0/1, xd=q2/3
GROUP_CHUNKS = 0                     # chunks per stream-run group; 0 = GATHER_BUFS-1
MERGE_SRC = 0                        # merge xs gathers of N buckets sharing s_tab (0/2)

ALIGN512 = True  # 512 B gather descriptors (2 rows/desc, parity-split buckets)
SUPER = 65536    # rows reachable by int16 pair-tokens (32768 x 512 B)
N_SUPER = 2      # ceil(N_NODES / SUPER)

_CACHE = {}
LAST_RESULTS = None  # bass_utils.BassKernelResults from the most recent run


def _build_nc(caps, cols_max, body_reps=1):
    """caps[r][k] = padded bucket capacity (multiple of 128) for relation r,
    bucket k = src_range*N_RANGES + dst_range. cols_max = max_r total cols."""
    from contextlib import ExitStack

    import concourse.bacc as bacc
    import concourse.mybir as mybir
    import concourse.tile as tile

    tot_s = sum(sum(c) for c in caps) // 16  # int16 image cols per side
    dt_g = getattr(mybir.dt, GATHER_DTYPE)
    assert not MERGE_SRC or REL_FOLDED  # merge path has no relb multiply

    nc = bacc.Bacc("TRN2", target_bir_lowering=False, debug=False,
                   num_devices=N_CORES, num_swdge_queues=N_QUEUES)
    table = nc.dram_tensor("table", (N_NODES, DIM), dt_g,
                           kind="ExternalInput")
    if REL_FOLDED:
        tabr = nc.dram_tensor("tabr", (N_REL, N_NODES, DIM), dt_g,
                              kind="ExternalInput")
    else:
        relb = nc.dram_tensor("relb", (N_REL, 128, DIM), dt_g,
                              kind="ExternalInput")
    sidx = nc.dram_tensor("sidx", (128, tot_s), mybir.dt.int16,
                          kind="ExternalInput")
    didx = nc.dram_tensor("didx", (128, tot_s), mybir.dt.int16,
                          kind="ExternalInput")
    out = nc.dram_tensor("out", (N_REL, 128, cols_max), mybir.dt.float32,
                         kind="ExternalOutput")
    qn = [0]  # round-robin SWDGE queue assignment state

    with tile.TileContext(nc) as tc, ExitStack() as ctx:
        const_pool = ctx.enter_context(tc.tile_pool(name="const", bufs=2))
        ipool = ctx.enter_context(tc.tile_pool(name="idx", bufs=1))
        if MERGE_SRC:
            xpool = ctx.enter_context(tc.tile_pool(name="xsm", bufs=5))
            dpool = ctx.enter_context(tc.tile_pool(name="xdm", bufs=7))
        else:
            gpool = ctx.enter_context(tc.tile_pool(name="gather",
                                                   bufs=GATHER_BUFS))
        spool = ctx.enter_context(tc.tile_pool(name="scores", bufs=2))

        def next_q(side=None):
            if QUEUE_MODE == "side" and N_QUEUES == 4 and side is not None:
                base = 0 if side == "s" else 2
                q = base + qn[0] % 2
                qn[0] += 1
                return q
            q = qn[0]
            qn[0] = (q + 1) % N_QUEUES
            return q

        # whole int16 index images stay SBUF-resident (~3 MB per side)
        sidx_sb = ipool.tile([128, tot_s], mybir.dt.int16, tag="sidx")
        didx_sb = ipool.tile([128, tot_s], mybir.dt.int16, tag="didx")
        nc.sync.dma_start(out=sidx_sb[:], in_=sidx[:, :])
        nc.sync.dma_start(out=didx_sb[:], in_=didx[:, :])

        for _rep in range(body_reps):
            img_off = 0  # running column offset into the int16 index images
            for r in range(N_REL):
                if not REL_FOLDED:
                    relb_sb = const_pool.tile([128, DIM], dt_g, tag="relb")
                    nc.sync.dma_start(out=relb_sb[:], in_=relb[r, :, :])
                cols_r = sum(caps[r]) // 128
                scores_sb = spool.tile([128, cols_max], mybir.dt.float32,
                                       tag="scores")
                # flatten the chunk list across buckets so the stream-run
                # grouping below actually gets GATHER_BUFS-1 chunks per
                # group (a single bucket is only ~31 cols = 1 chunk)
                col_off = 0
                chunks = []
                for k in range(N_BUCKETS):
                    cap = caps[r][k]
                    if cap == 0:
                        continue
                    if ALIGN512:
                        ks, sp = k >> 3, (k >> 2) & 1
                        kd, dp = (k >> 1) & 1, k & 1
                        ssi, dsi = ks * SUPER, kd * SUPER
                        s_base = (tabr[r, ssi : min(ssi + SUPER, N_NODES), :]
                                  if REL_FOLDED else
                                  table[ssi : min(ssi + SUPER, N_NODES), :])
                        s_tab = s_base.rearrange("(t e) d -> t (e d)", e=2)
                        d_tab = table[dsi : min(dsi + SUPER, N_NODES), :] \
                            .rearrange("(t e) d -> t (e d)", e=2)
                    else:
                        sp = dp = 0
                        si = (k // N_RANGES) * RANGE
                        di = (k % N_RANGES) * RANGE
                        if REL_FOLDED:
                            s_tab = tabr[r, si : min(si + RANGE, N_NODES), :]
                        else:
                            s_tab = table[si : min(si + RANGE, N_NODES), :]
                        d_tab = table[di : min(di + RANGE, N_NODES), :]
                    cols_b = cap // 128
                    for c0 in range(0, cols_b, CHUNK_COLS):
                        cc = min(CHUNK_COLS, cols_b - c0)
                        chunks.append((cc, img_off, s_tab, d_tab, col_off,
                                       sp, dp))
                        img_off += cc * 8
                        col_off += cc
                assert col_off == cols_r
                # issue gathers grouped by stream (src runs, then dst runs):
                # back-to-back same-stream gathers beat alternating src/dst.
                # Group size capped below GATHER_BUFS so the xs tiles of a
                # group can all be live at once.
                if MERGE_SRC:
                    # merge consecutive chunks sharing s_tab into one xs
                    # gather (<= 2*CHUNK_COLS cols = 8192 descs, HW-safe)
                    merged = []
                    for ch in chunks:
                        cc, io, s_tab, d_tab, co = ch
                        if (merged and merged[-1][2] is s_tab
                                and merged[-1][0] + cc <= 2 * CHUNK_COLS):
                            m = merged[-1]
                            merged[-1] = (m[0] + cc, m[1], s_tab, m[3] + [ch])
                        else:
                            merged.append((cc, io, s_tab, [ch]))
                    for g0 in range(0, len(merged), 4):
                        mg = merged[g0 : g0 + 4]
                        xs_tiles = []
                        for cct, io0, s_tab, subs in mg:
                            xs_t = xpool.tile([128, 2 * CHUNK_COLS * DIM],
                                              dt_g, tag="xsm")
                            nc.gpsimd.dma_gather(
                                xs_t[:, : cct * DIM].rearrange(
                                    "p (c d) -> p c d", d=DIM),
                                s_tab, sidx_sb[:, io0 : io0 + cct * 8],
                                cct * 128, cct * 128, DIM,
                                single_packet=SINGLE_PACKET,
                                queue_num=next_q("s"))
                            xs_tiles.append(xs_t)
                        for j, (cct, io0, s_tab, subs) in enumerate(mg):
                            xs_t = xs_tiles[j]
                            for cc, io, _st, d_tab, co in subs:
                                loc = (io - io0) // 8
                                xd_t = dpool.tile([128, CHUNK_COLS * DIM],
                                                  dt_g, tag="xd")
                                nc.gpsimd.dma_gather(
                                    xd_t[:, : cc * DIM].rearrange(
                                        "p (c d) -> p c d", d=DIM),
                                    d_tab, didx_sb[:, io : io + cc * 8],
                                    cc * 128, cc * 128, DIM,
                                    single_packet=SINGLE_PACKET,
                                    queue_num=next_q("d"))
                                nc.vector.tensor_tensor(
                                    out=xd_t[:, : cc * DIM],
                                    in0=xs_t[:, loc * DIM : (loc + cc) * DIM],
                                    in1=xd_t[:, : cc * DIM],
                                    op=mybir.AluOpType.mult)
                                nc.vector.reduce_sum(
                                    out=scores_sb[:, co : co + cc],
                                    in_=xd_t[:, : cc * DIM].rearrange(
                                        "p (c d) -> p c d", d=DIM),
                                    axis=mybir.AxisListType.X)
                    nc.sync.dma_start(out=out[r, :, :cols_r],
                                      in_=scores_sb[:, :cols_r])
                    continue
                EW = 2 * DIM if ALIGN512 else DIM  # gather elem width
                gsz = GROUP_CHUNKS or (GATHER_BUFS - 1)
                for g0 in range(0, len(chunks), gsz):
                    grp = chunks[g0 : g0 + gsz]
                    xs_tiles = []
                    for cc, io, s_tab, d_tab, co, sp, dp in grp:
                        xs_t = gpool.tile([128, CHUNK_COLS * EW],
                                          dt_g, tag="xs")
                        nc.gpsimd.dma_gather(
                            xs_t[:, : cc * EW].rearrange(
                                "p (c d) -> p c d", d=EW),
                            s_tab, sidx_sb[:, io : io + cc * 8],
                            cc * 128, cc * 128, EW,
                            single_packet=SINGLE_PACKET,
                            queue_num=next_q("s"))
                        xs_tiles.append(xs_t)
                    for j, (cc, io, s_tab, d_tab, co, sp, dp) in enumerate(grp):
                        xd_t = gpool.tile([128, CHUNK_COLS * EW],
                                          dt_g, tag="xd")
                        nc.gpsimd.dma_gather(
                            xd_t[:, : cc * EW].rearrange(
                                "p (c d) -> p c d", d=EW),
                            d_tab, didx_sb[:, io : io + cc * 8],
                            cc * 128, cc * 128, EW,
                            single_packet=SINGLE_PACKET,
                            queue_num=next_q("d"))
                        xs_t = xs_tiles[j]
                        xs3 = xs_t[:, : cc * EW].rearrange(
                            "p (c d) -> p c d", d=EW)
                        xd3 = xd_t[:, : cc * EW].rearrange(
                            "p (c d) -> p c d", d=EW)
                        prod = xs3[:, :, 0:DIM]
                        nc.vector.tensor_tensor(
                            out=prod,
                            in0=xs3[:, :, sp * DIM : (sp + 1) * DIM],
                            in1=xd3[:, :, dp * DIM : (dp + 1) * DIM],
                            op=mybir.AluOpType.mult)
                        if not REL_FOLDED:
                            nc.vector.tensor_tensor(
                                out=prod,
                                in0=prod,
                                in1=relb_sb[:, None, :].to_broadcast(
                                    [128, cc, DIM]),
                                op=mybir.AluOpType.mult)
                        nc.vector.reduce_sum(
                            out=scores_sb[:, co : co + cc],
                            in_=prod,
                            axis=mybir.AxisListType.X)
                nc.sync.dma_start(out=out[r, :, :cols_r],
                                  in_=scores_sb[:, :cols_r])

    nc.compile()
    return nc


def _pack_idx16(vals, cap):
    """Local indices [n] (n <= cap, cap % 128 == 0) -> replicated int16
    image [128, cap // 16], zero-padded."""
    a = np.zeros(cap, dtype=np.int16)
    a[: len(vals)] = vals
    return np.tile(a.reshape(-1, 16).T, (8, 1))  # [16, cap/16] -> [128, .]


SORT_MODE = "dst"  # none | src | dst | src_block_dst


def _bucket_order(s, d, b):
    """Permutation of edges grouped by bucket id b, with optional intra-bucket
    ordering for DRAM row-buffer locality on the gather streams."""
    if SORT_MODE == "none":
        return np.argsort(b, kind="stable")
    if SORT_MODE == "src":
        return np.lexsort((s, b))
    if SORT_MODE == "dst":
        return np.lexsort((d, b))
    if SORT_MODE == "dst_src":
        return np.lexsort((s, d, b))
    if SORT_MODE == "src_block_dst":
        BS = 4096  # src block rows
        return np.lexsort((d, s // BS, b))
    raise ValueError(SORT_MODE)


def prepare(node_embeds, rel_emb, src_idx, dst_idx, body_reps=1):
    """Bucket/pack host-side; returns (nc, in_maps, assemble) where
    assemble(results) -> full [N_REL, N_EDGES] scores."""
    host_dt = np.float16 if GATHER_DTYPE == "float16" else np.float32
    node32 = np.asarray(node_embeds, dtype=np.float32)
    rel32 = np.asarray(rel_emb, dtype=np.float32)
    node_embeds = np.ascontiguousarray(node32.astype(host_dt))
    rel_emb = rel32.astype(host_dt)
    src_idx = np.asarray(src_idx).astype(np.int64)
    dst_idx = np.asarray(dst_idx).astype(np.int64)

    if REL_FOLDED:
        tabr = np.ascontiguousarray(
            (node32[None, :, :] * rel32[:, None, :]).astype(host_dt))
    else:
        relb = np.ascontiguousarray(
            np.broadcast_to(rel_emb[:, None, :], (N_REL, 128, DIM)))

    # ---- host-side bucketing ----
    # orders[c][r]: edge permutation (bucket-major); counts[c][r][k]
    orders = [[None] * N_REL for _ in range(N_CORES)]
    counts = np.zeros((N_CORES, N_REL, N_BUCKETS), dtype=np.int64)
    s_loc = [[None] * N_REL for _ in range(N_CORES)]
    d_loc = [[None] * N_REL for _ in range(N_CORES)]
    for c in range(N_CORES):
        lo = c * E_CORE
        for r in range(N_REL):
            s = src_idx[r, lo : lo + E_CORE]
            d = dst_idx[r, lo : lo + E_CORE]
            if ALIGN512:
                b = ((s // SUPER) * 8 + (s % 2) * 4
                     + (d // SUPER) * 2 + (d % 2))
            else:
                b = (s // RANGE) * N_RANGES + (d // RANGE)
            order = _bucket_order(s, d, b)
            orders[c][r] = order
            counts[c, r] = np.bincount(b, minlength=N_BUCKETS)
            if ALIGN512:
                s_loc[c][r] = ((s[order] % SUPER) // 2).astype(np.int16)
                d_loc[c][r] = ((d[order] % SUPER) // 2).astype(np.int16)
            else:
                s_loc[c][r] = (s[order] % RANGE).astype(np.int16)
                d_loc[c][r] = (d[order] % RANGE).astype(np.int16)

    caps = [[int(-(-counts[:, r, k].max() // 128) * 128)
             for k in range(N_BUCKETS)] for r in range(N_REL)]
    cols_max = max(sum(caps[r]) for r in range(N_REL)) // 128

    key = (tuple(map(tuple, caps)), cols_max, body_reps, N_QUEUES,
           SINGLE_PACKET, GATHER_DTYPE, CHUNK_COLS, GATHER_BUFS, REL_FOLDED,
           QUEUE_MODE, GROUP_CHUNKS, MERGE_SRC, ALIGN512)
    if _CACHE.get("key") != key:
        _CACHE["nc"] = _build_nc(caps, cols_max, body_reps=body_reps)
        _CACHE["key"] = key
    nc = _CACHE["nc"]

    # ---- pack index images (chunked exactly like the device loop) ----
    tot_s = sum(sum(c) for c in caps) // 16
    in_maps = []
    for c in range(N_CORES):
        s_img = np.empty((128, tot_s), dtype=np.int16)
        d_img = np.empty((128, tot_s), dtype=np.int16)
        img_off = 0
        for r in range(N_REL):
            u = 0  # position within this core's bucket-sorted edge stream
            for k in range(N_BUCKETS):
                cap = caps[r][k]
                if cap == 0:
                    continue
                cnt = int(counts[c, r, k])
                sv = s_loc[c][r][u : u + cnt]
                dv = d_loc[c][r][u : u + cnt]
                u += cnt
                # pad bucket to cap, then emit in CHUNK_COLS chunks
                sp = np.zeros(cap, dtype=np.int16); sp[:cnt] = sv
                dp = np.zeros(cap, dtype=np.int16); dp[:cnt] = dv
                for c0 in range(0, cap // 128, CHUNK_COLS):
                    cc = min(CHUNK_COLS, cap // 128 - c0)
                    n = cc * 128
                    seg = slice(c0 * 128, c0 * 128 + n)
                    s_img[:, img_off : img_off + cc * 8] = _pack_idx16(sp[seg], n)
                    d_img[:, img_off : img_off + cc * 8] = _pack_idx16(dp[seg], n)
                    img_off += cc * 8
        assert img_off == tot_s
        m = {"table": node_embeds, "sidx": s_img, "didx": d_img}
        if REL_FOLDED:
            m["tabr"] = tabr
        else:
            m["relb"] = relb
        in_maps.append(m)

    def assemble(results):
        out = np.empty((N_REL, N_EDGES), dtype=np.float32)
        for c, res in enumerate(results):
            buf = res["out"]  # [N_REL, 128, cols_max]
            lo = c * E_CORE
            for r in range(N_REL):
                colmajor = buf[r].T.ravel()  # index = col*128 + partition
                u = 0
                off = 0
                for k in range(N_BUCKETS):
                    cap = caps[r][k]
                    if cap == 0:
                        continue
                    cnt = int(counts[c, r, k])
                    e_ids = orders[c][r][u : u + cnt]
                    out[r, lo + e_ids] = colmajor[off : off + cnt]
                    u += cnt
                    off += cap
        return out

    return nc, in_maps, assemble


def _spot_check(out, node_embeds, rel_emb, src_idx, dst_idx, n=256):
    """Recompute n random edges on the host (fp32) and count gross
    mismatches. Catches the rare multi-queue DMA-sem race (one nan
    warmup in ~31 runs observed): corrupted gathers read as wildly
    wrong or non-finite scores, far outside fp16 rounding (~1%)."""
    if not np.isfinite(out).all():
        return False
    rng = np.random.default_rng(0)
    x = np.asarray(node_embeds, dtype=np.float32)
    rel = np.asarray(rel_emb, dtype=np.float32)
    r = rng.integers(0, N_REL, n)
    e = rng.integers(0, N_EDGES, n)
    s = np.asarray(src_idx)[r, e].astype(np.int64)
    d = np.asarray(dst_idx)[r, e].astype(np.int64)
    exp = np.einsum("nd,nd,nd->n", x[s], x[d], rel[r])
    got = out[r, e]
    bad = np.abs(got - exp) > 0.2 * (np.abs(exp) + 1.0)
    return bad.mean() < 0.05


def kernel(node_embeds, rel_emb, src_idx, dst_idx):
    global LAST_RESULTS
    from concourse import bass_utils

    nc, in_maps, assemble = prepare(node_embeds, rel_emb, src_idx, dst_idx)
    for _attempt in range(3):
        LAST_RESULTS = bass_utils.run_bass_kernel_spmd(
            nc, in_maps, core_ids=list(range(N_CORES)))
        out = assemble(LAST_RESULTS.results)
        if _spot_check(out, node_embeds, rel_emb, src_idx, dst_idx):
            return out
    return out

